# revision 40
# baseline (speedup 1.0000x reference)
"""Distributed Trainium2 Bass kernel for nn_Attention_57346403336225.

Reference computation (per batch b of 16, N=1024 tokens, E=128 emb, H=8 heads,
head dim d = E = 128, INNER = 1024):
    xn   = LayerNorm(x) * ln_w + ln_b
    qkv  = xn @ w_qkv ; q,k,v heads
    dots = (q @ k^T) * scale[h]  ; diagonal masked to -FLT_MAX
    attn = softmax(dots) ; out = attn @ v
    y    = out @ w_proj + b_proj

Sharding: pure data-parallel over batch (16 / 8 cores = 2 batches per core),
weights replicated, no collectives.

Per-core algorithm (all heads/batches looped on-chip):
  - host precomputes A_h = scale[h] * Wq_h @ Wk_h^T  [E,E]  so that
    dots_h = xn @ A_h @ xn^T  (one [E,E] matmul replaces separate q,k)
  - LayerNorm in [tok, E] layout (bn_stats), PE-transpose -> xnT [E, tok]
  - v for all heads per batch via xn @ Wv with N=512 matmuls
  - dots tiles [128q, 1024k] in PSUM (bf16 matmuls), diagonal mask added on
    DVE, exp on ScalarE with fused per-row accumulation (softmax denom),
    P stored bf16
  - P^T via DMA xbar transposes (bf16) on the Sync engine (all other DMAs
    go through SWDGE so the xbar path owns Sync)
  - out^T accumulation over k chunks (bf16); projection per head with the
    softmax normalization (1/rowsum) applied as a per-partition scale in
    the projection epilogue, accumulated over heads on DVE.
"""

import numpy as np
import ml_dtypes

B, N, E, H = 16, 1024, 128, 8
NCORES = 8
B_LOC = B // NCORES  # 2
LN_EPS = 1e-5
NT = N // 128    # 8 token tiles per batch
MASK_VAL = -1e30

_cache = {}


def _build_nc():
    import concourse.bacc as bacc
    import concourse.mybir as mybir
    import concourse.tile as tile

    f32 = mybir.dt.float32
    bf16 = mybir.dt.bfloat16
    Exp = mybir.ActivationFunctionType.Exp
    Sqrt = mybir.ActivationFunctionType.Sqrt
    sub = mybir.AluOpType.subtract
    mult = mybir.AluOpType.mult
    add = mybir.AluOpType.add

    nc = bacc.Bacc("TRN2", target_bir_lowering=False)

    x_p = nc.declare_dram_parameter("x", [B_LOC, N, E], f32, isOutput=False)
    a_p = nc.declare_dram_parameter("amat", [H, E, E], bf16, isOutput=False)
    wvf_p = nc.declare_dram_parameter("wvf", [E, H * E], bf16, isOutput=False)
    wp_p = nc.declare_dram_parameter("wp", [H, E, E], bf16, isOutput=False)
    lnw_p = nc.declare_dram_parameter("lnw", [E, 1], f32, isOutput=False)
    lnb_p = nc.declare_dram_parameter("lnb", [E, 1], f32, isOutput=False)
    id_p = nc.declare_dram_parameter("ident", [E, E], f32, isOutput=False)
    dm_p = nc.declare_dram_parameter("dmask", [E, E], f32, isOutput=False)
    bp_p = nc.declare_dram_parameter("bptile", [128, E], f32, isOutput=False)
    out_p = nc.declare_dram_parameter("out", [B_LOC, N, E], f32, isOutput=True)

    with tile.TileContext(nc) as tc:
        with (
            tc.tile_pool(name="const", bufs=1) as cpool,
            tc.tile_pool(name="ln", bufs=8) as lnpool,
            tc.tile_pool(name="work", bufs=4) as wpool,
            tc.tile_pool(name="bigP", bufs=3) as ppool,
            tc.tile_pool(name="bigPT", bufs=4) as ptpool,
            tc.tile_pool(name="psd", bufs=2, space="PSUM") as psd,
            tc.tile_pool(name="psm", bufs=3, space="PSUM") as psm,
            tc.tile_pool(name="psw", bufs=1, space="PSUM") as psw,
        ):
            # ---- constants ----
            ident = cpool.tile([E, E], f32, tag="ident")
            dmask = cpool.tile([E, E], f32, tag="dmask")
            lnw = cpool.tile([E, 1], f32, tag="lnw")
            lnb = cpool.tile([E, 1], f32, tag="lnb")
            amat = cpool.tile([E, H, E], bf16, tag="amat")
            wvf = cpool.tile([E, H * E], bf16, tag="wvf")
            wp = cpool.tile([E, H, E], bf16, tag="wp")
            nc.sync.dma_start(ident[:], id_p[:])
            nc.gpsimd.dma_start(dmask[:], dm_p[:])
            nc.sync.dma_start(lnw[:], lnw_p[:])
            nc.sync.dma_start(lnb[:], lnb_p[:])
            nc.gpsimd.dma_start(amat[:], a_p[:].rearrange("h a b -> a h b"))
            nc.gpsimd.dma_start(wvf[:], wvf_p[:])
            nc.gpsimd.dma_start(wp[:], wp_p[:].rearrange("h a b -> a h b"))
            epst = cpool.tile([128, 1], f32, tag="epst")
            nc.vector.memset(epst[:], LN_EPS)
            zbias = cpool.tile([128, 1], f32, tag="zbias")
            nc.vector.memset(zbias[:], 0.0)
            bptile = cpool.tile([128, E], f32, tag="bptile")
            nc.gpsimd.dma_start(bptile[:], bp_p[:])

            # ---- LayerNorm + transpose -> xnT (bf16); v for all heads ----
            xnT = [cpool.tile([E, N], bf16, tag=f"xnT{b}", name=f"xnT{b}")
                   for b in range(B_LOC)]
            vall = [cpool.tile([128, NT, H * E], bf16, tag=f"vall{b}",
                               name=f"vall{b}") for b in range(B_LOC)]
            y_acc = [cpool.tile([128, N], f32, tag=f"yacc{b}", name=f"yacc{b}")
                     for b in range(B_LOC)]

            def ln_only(b, t):
                """LayerNorm tile t of batch b, transposed into xnT."""
                xt = lnpool.tile([128, E], f32, tag="xt", name=f"xt{b}_{t}")
                nc.sync.dma_start(xt[:], x_p[b, t * 128:(t + 1) * 128, :])
                st = lnpool.tile([128, 6], f32, tag="st", name=f"st{b}_{t}")
                nc.vector.bn_stats(st[:], xt[:])
                mv = lnpool.tile([128, 2], f32, tag="mv", name=f"mv{b}_{t}")
                nc.vector.bn_aggr(mv[:], st[:])
                sd = lnpool.tile([128, 1], f32, tag="sd", name=f"sd{b}_{t}")
                nc.scalar.activation(sd[:], mv[:, 1:2], Sqrt, bias=epst[:])
                rs = lnpool.tile([128, 1], f32, tag="rs", name=f"rs{b}_{t}")
                nc.vector.reciprocal(rs[:], sd[:])
                xn = lnpool.tile([128, E], f32, tag="xnt", name=f"xn{b}_{t}")
                nc.vector.tensor_scalar(
                    xn[:], xt[:], mv[:, 0:1], rs[:], op0=sub, op1=mult
                )
                tp = psm.tile([128, E], f32, tag="m512", name=f"lntp{b}_{t}")
                nc.tensor.transpose(tp[:], xn[:], ident[:])
                nc.vector.tensor_scalar(
                    xnT[b][:, t * 128:(t + 1) * 128], tp[:],
                    lnw[:], lnb[:], op0=mult, op1=add,
                )
            def v_part(b, t):
                for c in range(2):
                    vps = psm.tile([128, 512], f32, tag="m512",
                                   name=f"vps{b}_{t}_{c}")
                    nc.tensor.matmul(
                        vps[:],
                        xnT[b][:, t * 128:(t + 1) * 128],
                        wvf[:, c * 512:(c + 1) * 512],
                        start=True, stop=True,
                    )
                    nc.vector.tensor_copy(
                        vall[b][:, t, c * 512:(c + 1) * 512], vps[:]
                    )

            def ln_and_v(b, t):
                ln_only(b, t)
                v_part(b, t)

            for t in range(NT):
                ln_only(0, t)

            # ---- attention, software-pipelined across (batch, head) ----
            # 3-deep pipeline, emission order per step `it`:
            #   tT(it+1) early (its DVE cast settles before step it+1)
            #   dots/mask/exp/transpose(it) interleaved with PV(it-1)
            #   proj(it-2) last (oT cast from step it-1 has settled)
            iters = [(b, h) for b in range(B_LOC) for h in range(H)]
            NIT = len(iters)
            stash = {}
            warm_ps = psw.tile([64, 64], f32, tag="warm")

            def warm(n=2):
                for _ in range(n):
                    nc.tensor.matmul(
                        warm_ps[:], amat[:, 0, :64], wvf[:, :64],
                        start=True, stop=True, skip_group_check=True,
                    )

            def make_tT(it):
                b, h = iters[it]
                tT = wpool.tile([E, N], bf16, tag="tT", name=f"tT{it}")
                for qc in range(2):
                    tps = psm.tile([128, 512], f32, tag="m512", name=f"tps{it}_{qc}")
                    nc.tensor.matmul(
                        tps[:], amat[:, h, :],
                        xnT[b][:, qc * 512:(qc + 1) * 512],
                        start=True, stop=True,
                    )
                    nc.vector.tensor_copy(tT[:, qc * 512:(qc + 1) * 512], tps[:])
                stash[("tT", it)] = tT

            def dots_group(it, qts, P, rsum, PT):
                b, h = iters[it]
                tT = stash[("tT", it)]
                for qt in qts:
                    dps = psd.tile([128, N], f32, tag="dots", name=f"dps{it}_{qt}")
                    for kc in range(2):
                        nc.tensor.matmul(
                            dps[:, kc * 512:(kc + 1) * 512],
                            tT[:, qt * 128:(qt + 1) * 128],
                            xnT[b][:, kc * 512:(kc + 1) * 512],
                            start=True, stop=True,
                        )
                    nc.vector.tensor_add(
                        dps[:, qt * 128:(qt + 1) * 128],
                        dps[:, qt * 128:(qt + 1) * 128],
                        dmask[:],
                    )
                    nc.scalar.activation(
                        P[:, qt, :], dps[:], Exp, bias=zbias[:],
                        accum_out=rsum[:, qt:qt + 1],
                    )
                    nc.sync.dma_start(
                        out=PT[:, :, qt * 128:(qt + 1) * 128],
                        in_=P[:, qt, :],
                        transpose=True,
                    )

            def pv_chain(it, qc):
                b, h = iters[it]
                PT = stash[("PT", it)]
                oT = stash[("oT", it)]
                ops = psm.tile([128, 512], f32, tag="m512", name=f"ops{it}_{qc}")
                for kt in range(NT):
                    nc.tensor.matmul(
                        ops[:],
                        vall[b][:, kt, h * E:(h + 1) * E],
                        PT[:, kt, qc * 512:(qc + 1) * 512],
                        start=(kt == 0), stop=(kt == NT - 1),
                    )
                nc.vector.tensor_copy(oT[:, qc * 512:(qc + 1) * 512], ops[:])

            def proj(it):
                b, h = iters[it]
                oT = stash.pop(("oT", it))
                rcp = stash.pop(("rcp", it))
                stash.pop(("tT", it))
                stash.pop(("PT", it))
                for t in range(NT):
                    yps = psm.tile([128, E], f32, tag="m512", name=f"yps{it}_{t}")
                    nc.tensor.matmul(
                        yps[:],
                        oT[:, t * 128:(t + 1) * 128],
                        wp[:, h, :],
                        start=True, stop=True,
                    )
                    if h == 0:
                        nc.vector.scalar_tensor_tensor(
                            y_acc[b][:, t * 128:(t + 1) * 128],
                            yps[:], rcp[:, t:t + 1], bptile[:],
                            op0=mult, op1=add,
                        )
                    else:
                        nc.vector.scalar_tensor_tensor(
                            y_acc[b][:, t * 128:(t + 1) * 128],
                            yps[:], rcp[:, t:t + 1],
                            y_acc[b][:, t * 128:(t + 1) * 128],
                            op0=mult, op1=add,
                        )
                if h == H - 1:
                    for t2 in range(0, NT, 2):
                        nc.scalar.dma_start(
                            out_p[b, t2 * 128:(t2 + 2) * 128].rearrange(
                                "(t p) e -> p t e", p=128),
                            y_acc[b][:, t2 * 128:(t2 + 2) * 128].rearrange(
                                "p (t e) -> p t e", t=2),
                        )


            make_tT(0)
            for it in range(NIT + 2):
                cur = it if it < NIT else None
                if cur is not None:
                    b, h = iters[cur]
                    P = ppool.tile([128, NT, N], bf16, tag="P", name=f"P{cur}")
                    rsum = wpool.tile([128, NT], f32, tag="rsum", name=f"rsum{cur}")
                    PT = ptpool.tile([128, NT, N], bf16, tag="PT", name=f"PT{cur}")
                    oT = wpool.tile([E, N], bf16, tag="oT", name=f"oT{cur}")
                    stash[("PT", cur)] = PT
                    stash[("oT", cur)] = oT
                    dots_group(cur, [0, 1, 2, 3], P, rsum, PT)
                    warm()
                if it == 0:
                    for _t in range(4):
                        v_part(0, _t)
                if it - 1 >= 0 and it - 1 < NIT:
                    pv_chain(it - 1, 0)
                if cur is not None:
                    dots_group(cur, [4, 5, 6, 7], P, rsum, PT)
                    warm()
                if it == 0:
                    for _t in range(4, NT):
                        v_part(0, _t)
                if it - 1 >= 0 and it - 1 < NIT:
                    pv_chain(it - 1, 1)
                    warm()
                if cur is not None:
                    rcp = wpool.tile([128, NT], f32, tag="rcp", name=f"rcp{cur}")
                    nc.vector.reciprocal(rcp[:], rsum[:])
                    stash[("rcp", cur)] = rcp
                if cur is not None and cur + 1 < NIT:
                    make_tT(cur + 1)
                if 1 <= it <= 4:
                    ln_and_v(1, (it - 1) * 2)
                    ln_and_v(1, (it - 1) * 2 + 1)
                if it - 2 >= 0 and it - 2 < NIT:
                    proj(it - 2)

    nc.compile()
    return nc


def _get_nc():
    if "nc" not in _cache:
        _cache["nc"] = _build_nc()
    return _cache["nc"]


def _make_in_maps(inputs):
    x = np.ascontiguousarray(np.asarray(inputs["x"], dtype=np.float32))
    ln_w = np.asarray(inputs["ln_w"], dtype=np.float32)
    ln_b = np.asarray(inputs["ln_b"], dtype=np.float32)
    w_qkv = np.asarray(inputs["w_qkv"], dtype=np.float32)
    scale = np.asarray(inputs["scale"], dtype=np.float32)
    w_proj = np.asarray(inputs["w_proj"], dtype=np.float32)

    INNER = E * H
    Wq = w_qkv[:, :INNER]
    Wk = w_qkv[:, INNER:2 * INNER]
    Wv = w_qkv[:, 2 * INNER:]

    amat = np.stack(
        [scale[h] * (Wq[:, h * E:(h + 1) * E] @ Wk[:, h * E:(h + 1) * E].T)
         for h in range(H)]
    ).astype(ml_dtypes.bfloat16)  # [H, E, E]
    wvf = Wv.astype(ml_dtypes.bfloat16)  # [E, INNER]
    wp = w_proj.reshape(H, E, E).astype(ml_dtypes.bfloat16)  # [H, d, E]
    ident = np.eye(E, dtype=np.float32)
    dmask = (np.eye(E, dtype=np.float32) * MASK_VAL).astype(np.float32)
    lnw = ln_w.reshape(E, 1)
    lnb = ln_b.reshape(E, 1)

    b_proj = np.asarray(inputs["b_proj"], dtype=np.float32)
    bptile = np.broadcast_to(b_proj[None, :], (128, E)).copy()
    shared = {
        "amat": amat, "wvf": wvf, "wp": wp, "bptile": bptile,
        "lnw": lnw, "lnb": lnb, "ident": ident, "dmask": dmask,
    }
    return [
        {"x": x[c * B_LOC:(c + 1) * B_LOC], **shared} for c in range(NCORES)
    ]


def kernel(x, ln_w, ln_b, w_qkv, scale, w_proj, b_proj):
    from concourse.bass_utils import run_bass_kernel_spmd

    in_maps = _make_in_maps(dict(
        x=x, ln_w=ln_w, ln_b=ln_b, w_qkv=w_qkv, scale=scale,
        w_proj=w_proj, b_proj=b_proj,
    ))
    b_proj = np.asarray(b_proj, dtype=np.float32)

    nc = _get_nc()
    res = run_bass_kernel_spmd(nc, in_maps, core_ids=list(range(NCORES)))
    y = np.concatenate([res.results[c]["out"] for c in range(NCORES)], axis=0)
    return y.astype(np.float32)


# revision 41
# speedup vs baseline: 1.0039x; 1.0039x over previous
"""Distributed Trainium2 Bass kernel for nn_Attention_57346403336225.

Reference computation (per batch b of 16, N=1024 tokens, E=128 emb, H=8 heads,
head dim d = E = 128, INNER = 1024):
    xn   = LayerNorm(x) * ln_w + ln_b
    qkv  = xn @ w_qkv ; q,k,v heads
    dots = (q @ k^T) * scale[h]  ; diagonal masked to -FLT_MAX
    attn = softmax(dots) ; out = attn @ v
    y    = out @ w_proj + b_proj

Sharding: pure data-parallel over batch (16 / 8 cores = 2 batches per core),
weights replicated, no collectives.

Per-core algorithm (all heads/batches looped on-chip):
  - host precomputes A_h = scale[h] * Wq_h @ Wk_h^T  [E,E]  so that
    dots_h = xn @ A_h @ xn^T  (one [E,E] matmul replaces separate q,k)
  - LayerNorm in [tok, E] layout (bn_stats), PE-transpose -> xnT [E, tok]
  - v for all heads per batch via xn @ Wv with N=512 matmuls
  - dots tiles [128q, 1024k] in PSUM (bf16 matmuls), diagonal mask added on
    DVE, exp on ScalarE with fused per-row accumulation (softmax denom),
    P stored bf16
  - P^T via DMA xbar transposes (bf16) on the Sync engine (all other DMAs
    go through SWDGE so the xbar path owns Sync)
  - out^T accumulation over k chunks (bf16); projection per head with the
    softmax normalization (1/rowsum) applied as a per-partition scale in
    the projection epilogue, accumulated over heads on DVE.
"""

import numpy as np
import ml_dtypes

B, N, E, H = 16, 1024, 128, 8
NCORES = 8
B_LOC = B // NCORES  # 2
LN_EPS = 1e-5
NT = N // 128    # 8 token tiles per batch
MASK_VAL = -1e30

_cache = {}


def _build_nc():
    import concourse.bacc as bacc
    import concourse.mybir as mybir
    import concourse.tile as tile

    f32 = mybir.dt.float32
    bf16 = mybir.dt.bfloat16
    Exp = mybir.ActivationFunctionType.Exp
    Sqrt = mybir.ActivationFunctionType.Sqrt
    sub = mybir.AluOpType.subtract
    mult = mybir.AluOpType.mult
    add = mybir.AluOpType.add

    nc = bacc.Bacc("TRN2", target_bir_lowering=False)

    x_p = nc.declare_dram_parameter("x", [B_LOC, N, E], f32, isOutput=False)
    a_p = nc.declare_dram_parameter("amat", [H, E, E], bf16, isOutput=False)
    wvf_p = nc.declare_dram_parameter("wvf", [E, H * E], bf16, isOutput=False)
    wp_p = nc.declare_dram_parameter("wp", [H, E, E], bf16, isOutput=False)
    lnw_p = nc.declare_dram_parameter("lnw", [E, 1], f32, isOutput=False)
    lnb_p = nc.declare_dram_parameter("lnb", [E, 1], f32, isOutput=False)
    id_p = nc.declare_dram_parameter("ident", [E, E], f32, isOutput=False)
    dm_p = nc.declare_dram_parameter("dmask", [E, E], f32, isOutput=False)
    bp_p = nc.declare_dram_parameter("bptile", [128, E], f32, isOutput=False)
    out_p = nc.declare_dram_parameter("out", [B_LOC, N, E], f32, isOutput=True)

    with tile.TileContext(nc) as tc:
        with (
            tc.tile_pool(name="const", bufs=1) as cpool,
            tc.tile_pool(name="ln", bufs=8) as lnpool,
            tc.tile_pool(name="work", bufs=4) as wpool,
            tc.tile_pool(name="bigP", bufs=3) as ppool,
            tc.tile_pool(name="bigPT", bufs=4) as ptpool,
            tc.tile_pool(name="psd", bufs=2, space="PSUM") as psd,
            tc.tile_pool(name="psm", bufs=3, space="PSUM") as psm,
            tc.tile_pool(name="psw", bufs=1, space="PSUM") as psw,
        ):
            # ---- constants ----
            ident = cpool.tile([E, E], f32, tag="ident")
            dmask = cpool.tile([E, E], f32, tag="dmask")
            lnw = cpool.tile([E, 1], f32, tag="lnw")
            lnb = cpool.tile([E, 1], f32, tag="lnb")
            amat = cpool.tile([E, H, E], bf16, tag="amat")
            wvf = cpool.tile([E, H * E], bf16, tag="wvf")
            wp = cpool.tile([E, H, E], bf16, tag="wp")
            nc.sync.dma_start(ident[:], id_p[:])
            nc.gpsimd.dma_start(dmask[:], dm_p[:])
            nc.sync.dma_start(lnw[:], lnw_p[:])
            nc.sync.dma_start(lnb[:], lnb_p[:])
            nc.gpsimd.dma_start(amat[:], a_p[:].rearrange("h a b -> a h b"))
            nc.gpsimd.dma_start(wvf[:], wvf_p[:])
            nc.gpsimd.dma_start(wp[:], wp_p[:].rearrange("h a b -> a h b"))
            epst = cpool.tile([128, 1], f32, tag="epst")
            nc.vector.memset(epst[:], LN_EPS)
            zbias = cpool.tile([128, 1], f32, tag="zbias")
            nc.vector.memset(zbias[:], 0.0)
            bptile = cpool.tile([128, E], f32, tag="bptile")
            nc.gpsimd.dma_start(bptile[:], bp_p[:])

            # ---- LayerNorm + transpose -> xnT (bf16); v for all heads ----
            xnT = [cpool.tile([E, N], bf16, tag=f"xnT{b}", name=f"xnT{b}")
                   for b in range(B_LOC)]
            vall = [cpool.tile([128, NT, H * E], bf16, tag=f"vall{b}",
                               name=f"vall{b}") for b in range(B_LOC)]
            y_acc = [cpool.tile([128, N], f32, tag=f"yacc{b}", name=f"yacc{b}")
                     for b in range(B_LOC)]

            def ln_and_v(b, t):
                """LayerNorm tile t of batch b, transpose into xnT, and the
                v-projection matmuls for that token tile (all heads)."""
                xt = lnpool.tile([128, E], f32, tag="xt", name=f"xt{b}_{t}")
                nc.sync.dma_start(xt[:], x_p[b, t * 128:(t + 1) * 128, :])
                st = lnpool.tile([128, 6], f32, tag="st", name=f"st{b}_{t}")
                nc.vector.bn_stats(st[:], xt[:])
                mv = lnpool.tile([128, 2], f32, tag="mv", name=f"mv{b}_{t}")
                nc.vector.bn_aggr(mv[:], st[:])
                sd = lnpool.tile([128, 1], f32, tag="sd", name=f"sd{b}_{t}")
                nc.scalar.activation(sd[:], mv[:, 1:2], Sqrt, bias=epst[:])
                rs = lnpool.tile([128, 1], f32, tag="rs", name=f"rs{b}_{t}")
                nc.vector.reciprocal(rs[:], sd[:])
                xn = lnpool.tile([128, E], f32, tag="xnt", name=f"xn{b}_{t}")
                nc.vector.tensor_scalar(
                    xn[:], xt[:], mv[:, 0:1], rs[:], op0=sub, op1=mult
                )
                tp = psm.tile([128, E], f32, tag="m512", name=f"lntp{b}_{t}")
                nc.tensor.transpose(tp[:], xn[:], ident[:])
                nc.vector.tensor_scalar(
                    xnT[b][:, t * 128:(t + 1) * 128], tp[:],
                    lnw[:], lnb[:], op0=mult, op1=add,
                )
                for c in range(2):
                    vps = psm.tile([128, 512], f32, tag="m512",
                                   name=f"vps{b}_{t}_{c}")
                    nc.tensor.matmul(
                        vps[:],
                        xnT[b][:, t * 128:(t + 1) * 128],
                        wvf[:, c * 512:(c + 1) * 512],
                        start=True, stop=True,
                    )
                    nc.vector.tensor_copy(
                        vall[b][:, t, c * 512:(c + 1) * 512], vps[:]
                    )

            for t in range(NT):
                ln_and_v(0, t)

            # ---- attention, software-pipelined across (batch, head) ----
            # 3-deep pipeline, emission order per step `it`:
            #   tT(it+1) early (its DVE cast settles before step it+1)
            #   dots/mask/exp/transpose(it) interleaved with PV(it-1)
            #   proj(it-2) last (oT cast from step it-1 has settled)
            iters = [(b, h) for b in range(B_LOC) for h in range(H)]
            NIT = len(iters)
            stash = {}
            warm_ps = psw.tile([64, 64], f32, tag="warm")

            def warm(n=2):
                for _ in range(n):
                    nc.tensor.matmul(
                        warm_ps[:], amat[:, 0, :64], wvf[:, :64],
                        start=True, stop=True, skip_group_check=True,
                    )

            def make_tT(it):
                b, h = iters[it]
                tT = wpool.tile([E, N], bf16, tag="tT", name=f"tT{it}")
                for qc in range(2):
                    tps = psm.tile([128, 512], f32, tag="m512", name=f"tps{it}_{qc}")
                    nc.tensor.matmul(
                        tps[:], amat[:, h, :],
                        xnT[b][:, qc * 512:(qc + 1) * 512],
                        start=True, stop=True,
                    )
                    nc.vector.tensor_copy(tT[:, qc * 512:(qc + 1) * 512], tps[:])
                stash[("tT", it)] = tT

            def dots_group(it, qts, P, rsum, PT):
                b, h = iters[it]
                tT = stash[("tT", it)]
                for qt in qts:
                    dps = psd.tile([128, N], f32, tag="dots", name=f"dps{it}_{qt}")
                    for kc in range(2):
                        nc.tensor.matmul(
                            dps[:, kc * 512:(kc + 1) * 512],
                            tT[:, qt * 128:(qt + 1) * 128],
                            xnT[b][:, kc * 512:(kc + 1) * 512],
                            start=True, stop=True,
                        )
                    nc.vector.tensor_add(
                        dps[:, qt * 128:(qt + 1) * 128],
                        dps[:, qt * 128:(qt + 1) * 128],
                        dmask[:],
                    )
                    nc.scalar.activation(
                        P[:, qt, :], dps[:], Exp, bias=zbias[:],
                        accum_out=rsum[:, qt:qt + 1],
                    )
                    nc.sync.dma_start(
                        out=PT[:, :, qt * 128:(qt + 1) * 128],
                        in_=P[:, qt, :],
                        transpose=True,
                    )

            def pv_chain(it, qc):
                b, h = iters[it]
                PT = stash[("PT", it)]
                oT = stash[("oT", it)]
                ops = psm.tile([128, 512], f32, tag="m512", name=f"ops{it}_{qc}")
                for kt in range(NT):
                    nc.tensor.matmul(
                        ops[:],
                        vall[b][:, kt, h * E:(h + 1) * E],
                        PT[:, kt, qc * 512:(qc + 1) * 512],
                        start=(kt == 0), stop=(kt == NT - 1),
                    )
                nc.vector.tensor_copy(oT[:, qc * 512:(qc + 1) * 512], ops[:])

            def proj(it):
                b, h = iters[it]
                oT = stash.pop(("oT", it))
                rcp = stash.pop(("rcp", it))
                stash.pop(("tT", it))
                stash.pop(("PT", it))
                for t in range(NT):
                    yps = psm.tile([128, E], f32, tag="m512", name=f"yps{it}_{t}")
                    nc.tensor.matmul(
                        yps[:],
                        oT[:, t * 128:(t + 1) * 128],
                        wp[:, h, :],
                        start=True, stop=True,
                    )
                    if h == 0:
                        nc.vector.scalar_tensor_tensor(
                            y_acc[b][:, t * 128:(t + 1) * 128],
                            yps[:], rcp[:, t:t + 1], bptile[:],
                            op0=mult, op1=add,
                        )
                    else:
                        nc.vector.scalar_tensor_tensor(
                            y_acc[b][:, t * 128:(t + 1) * 128],
                            yps[:], rcp[:, t:t + 1],
                            y_acc[b][:, t * 128:(t + 1) * 128],
                            op0=mult, op1=add,
                        )
                if h == H - 1:
                    for t2 in range(0, NT, 2):
                        nc.scalar.dma_start(
                            out_p[b, t2 * 128:(t2 + 2) * 128].rearrange(
                                "(t p) e -> p t e", p=128),
                            y_acc[b][:, t2 * 128:(t2 + 2) * 128].rearrange(
                                "p (t e) -> p t e", t=2),
                        )


            make_tT(0)
            for it in range(NIT + 2):
                cur = it if it < NIT else None
                if cur is not None:
                    b, h = iters[cur]
                    P = ppool.tile([128, NT, N], bf16, tag="P", name=f"P{cur}")
                    rsum = wpool.tile([128, NT], f32, tag="rsum", name=f"rsum{cur}")
                    PT = ptpool.tile([128, NT, N], bf16, tag="PT", name=f"PT{cur}")
                    oT = wpool.tile([E, N], bf16, tag="oT", name=f"oT{cur}")
                    stash[("PT", cur)] = PT
                    stash[("oT", cur)] = oT
                    dots_group(cur, [0, 1, 2, 3], P, rsum, PT)
                    warm()
                if it - 1 >= 0 and it - 1 < NIT:
                    pv_chain(it - 1, 0)
                if cur is not None:
                    dots_group(cur, [4, 5, 6, 7], P, rsum, PT)
                    warm()
                if it - 1 >= 0 and it - 1 < NIT:
                    pv_chain(it - 1, 1)
                    warm()
                if cur is not None:
                    rcp = wpool.tile([128, NT], f32, tag="rcp", name=f"rcp{cur}")
                    nc.vector.reciprocal(rcp[:], rsum[:])
                    stash[("rcp", cur)] = rcp
                if cur is not None and cur + 1 < NIT:
                    make_tT(cur + 1)
                if 1 <= it <= 4:
                    ln_and_v(1, (it - 1) * 2)
                    ln_and_v(1, (it - 1) * 2 + 1)
                if it - 2 >= 0 and it - 2 < NIT:
                    proj(it - 2)

    nc.compile()
    return nc


def _get_nc():
    if "nc" not in _cache:
        _cache["nc"] = _build_nc()
    return _cache["nc"]


def _make_in_maps(inputs):
    x = np.ascontiguousarray(np.asarray(inputs["x"], dtype=np.float32))
    ln_w = np.asarray(inputs["ln_w"], dtype=np.float32)
    ln_b = np.asarray(inputs["ln_b"], dtype=np.float32)
    w_qkv = np.asarray(inputs["w_qkv"], dtype=np.float32)
    scale = np.asarray(inputs["scale"], dtype=np.float32)
    w_proj = np.asarray(inputs["w_proj"], dtype=np.float32)

    INNER = E * H
    Wq = w_qkv[:, :INNER]
    Wk = w_qkv[:, INNER:2 * INNER]
    Wv = w_qkv[:, 2 * INNER:]

    amat = np.stack(
        [scale[h] * (Wq[:, h * E:(h + 1) * E] @ Wk[:, h * E:(h + 1) * E].T)
         for h in range(H)]
    ).astype(ml_dtypes.bfloat16)  # [H, E, E]
    wvf = Wv.astype(ml_dtypes.bfloat16)  # [E, INNER]
    wp = w_proj.reshape(H, E, E).astype(ml_dtypes.bfloat16)  # [H, d, E]
    ident = np.eye(E, dtype=np.float32)
    dmask = (np.eye(E, dtype=np.float32) * MASK_VAL).astype(np.float32)
    lnw = ln_w.reshape(E, 1)
    lnb = ln_b.reshape(E, 1)

    b_proj = np.asarray(inputs["b_proj"], dtype=np.float32)
    bptile = np.broadcast_to(b_proj[None, :], (128, E)).copy()
    shared = {
        "amat": amat, "wvf": wvf, "wp": wp, "bptile": bptile,
        "lnw": lnw, "lnb": lnb, "ident": ident, "dmask": dmask,
    }
    return [
        {"x": x[c * B_LOC:(c + 1) * B_LOC], **shared} for c in range(NCORES)
    ]


def kernel(x, ln_w, ln_b, w_qkv, scale, w_proj, b_proj):
    from concourse.bass_utils import run_bass_kernel_spmd

    in_maps = _make_in_maps(dict(
        x=x, ln_w=ln_w, ln_b=ln_b, w_qkv=w_qkv, scale=scale,
        w_proj=w_proj, b_proj=b_proj,
    ))
    b_proj = np.asarray(b_proj, dtype=np.float32)

    nc = _get_nc()
    res = run_bass_kernel_spmd(nc, in_maps, core_ids=list(range(NCORES)))
    y = np.concatenate([res.results[c]["out"] for c in range(NCORES)], axis=0)
    return y.astype(np.float32)


# revision 42
# speedup vs baseline: 1.0385x; 1.0345x over previous
"""Distributed Trainium2 Bass kernel for nn_Attention_57346403336225.

Reference computation (per batch b of 16, N=1024 tokens, E=128 emb, H=8 heads,
head dim d = E = 128, INNER = 1024):
    xn   = LayerNorm(x) * ln_w + ln_b
    qkv  = xn @ w_qkv ; q,k,v heads
    dots = (q @ k^T) * scale[h]  ; diagonal masked to -FLT_MAX
    attn = softmax(dots) ; out = attn @ v
    y    = out @ w_proj + b_proj

Sharding: pure data-parallel over batch (16 / 8 cores = 2 batches per core),
weights replicated, no collectives.

Per-core algorithm (all heads/batches looped on-chip):
  - host precomputes A_h = scale[h] * Wq_h @ Wk_h^T  [E,E]  so that
    dots_h = xn @ A_h @ xn^T  (one [E,E] matmul replaces separate q,k)
  - LayerNorm in [tok, E] layout (bn_stats), PE-transpose -> xnT [E, tok]
  - v for all heads per batch via xn @ Wv with N=512 matmuls
  - dots tiles [128q, 1024k] in PSUM (bf16 matmuls), diagonal mask added on
    DVE, exp on ScalarE with fused per-row accumulation (softmax denom),
    P stored bf16
  - P^T via DMA xbar transposes (bf16) on the Sync engine (all other DMAs
    go through SWDGE so the xbar path owns Sync)
  - out^T accumulation over k chunks (bf16); projection per head with the
    softmax normalization (1/rowsum) applied as a per-partition scale in
    the projection epilogue, accumulated over heads on DVE.
"""

import numpy as np
import ml_dtypes

B, N, E, H = 16, 1024, 128, 8
NCORES = 8
B_LOC = B // NCORES  # 2
LN_EPS = 1e-5
NT = N // 128    # 8 token tiles per batch
MASK_VAL = -1e30

_cache = {}


def _build_nc():
    import concourse.bacc as bacc
    import concourse.mybir as mybir
    import concourse.tile as tile

    f32 = mybir.dt.float32
    bf16 = mybir.dt.bfloat16
    Exp = mybir.ActivationFunctionType.Exp
    Sqrt = mybir.ActivationFunctionType.Sqrt
    sub = mybir.AluOpType.subtract
    mult = mybir.AluOpType.mult
    add = mybir.AluOpType.add

    nc = bacc.Bacc("TRN2", target_bir_lowering=False)

    x_p = nc.declare_dram_parameter("x", [B_LOC, N, E], f32, isOutput=False)
    a_p = nc.declare_dram_parameter("amat", [H, E, E], bf16, isOutput=False)
    wvf_p = nc.declare_dram_parameter("wvf", [E, H * E], bf16, isOutput=False)
    wp_p = nc.declare_dram_parameter("wp", [H, E, E], bf16, isOutput=False)
    lnw_p = nc.declare_dram_parameter("lnw", [E, 1], f32, isOutput=False)
    lnb_p = nc.declare_dram_parameter("lnb", [E, 1], f32, isOutput=False)
    id_p = nc.declare_dram_parameter("ident", [E, E], f32, isOutput=False)
    dm_p = nc.declare_dram_parameter("dmask", [E, E], f32, isOutput=False)
    bp_p = nc.declare_dram_parameter("bptile", [128, E], f32, isOutput=False)
    out_p = nc.declare_dram_parameter("out", [B_LOC, N, E], f32, isOutput=True)

    with tile.TileContext(nc) as tc:
        with (
            tc.tile_pool(name="const", bufs=1) as cpool,
            tc.tile_pool(name="ln", bufs=8) as lnpool,
            tc.tile_pool(name="work", bufs=4) as wpool,
            tc.tile_pool(name="bigP", bufs=3) as ppool,
            tc.tile_pool(name="bigPT", bufs=4) as ptpool,
            tc.tile_pool(name="psd", bufs=3, space="PSUM") as psd,
            tc.tile_pool(name="psm", bufs=2, space="PSUM") as psm,
        ):
            # ---- constants ----
            ident = cpool.tile([E, E], f32, tag="ident")
            dmask = cpool.tile([E, E], f32, tag="dmask")
            lnw = cpool.tile([E, 1], f32, tag="lnw")
            lnb = cpool.tile([E, 1], f32, tag="lnb")
            amat = cpool.tile([E, H, E], bf16, tag="amat")
            wvf = cpool.tile([E, H * E], bf16, tag="wvf")
            wp = cpool.tile([E, H, E], bf16, tag="wp")
            nc.sync.dma_start(ident[:], id_p[:])
            nc.gpsimd.dma_start(dmask[:], dm_p[:])
            nc.sync.dma_start(lnw[:], lnw_p[:])
            nc.sync.dma_start(lnb[:], lnb_p[:])
            nc.gpsimd.dma_start(amat[:], a_p[:].rearrange("h a b -> a h b"))
            nc.gpsimd.dma_start(wvf[:], wvf_p[:])
            nc.gpsimd.dma_start(wp[:], wp_p[:].rearrange("h a b -> a h b"))
            epst = cpool.tile([128, 1], f32, tag="epst")
            nc.vector.memset(epst[:], LN_EPS)
            zbias = cpool.tile([128, 1], f32, tag="zbias")
            nc.vector.memset(zbias[:], 0.0)
            bptile = cpool.tile([128, E], f32, tag="bptile")
            nc.gpsimd.dma_start(bptile[:], bp_p[:])

            # ---- LayerNorm + transpose -> xnT (bf16); v for all heads ----
            xnT = [cpool.tile([E, N], bf16, tag=f"xnT{b}", name=f"xnT{b}")
                   for b in range(B_LOC)]
            vall = [cpool.tile([128, NT, H * E], bf16, tag=f"vall{b}",
                               name=f"vall{b}") for b in range(B_LOC)]
            y_acc = [cpool.tile([128, N], f32, tag=f"yacc{b}", name=f"yacc{b}")
                     for b in range(B_LOC)]

            def ln_and_v(b, t):
                """LayerNorm tile t of batch b, transpose into xnT, and the
                v-projection matmuls for that token tile (all heads)."""
                xt = lnpool.tile([128, E], f32, tag="xt", name=f"xt{b}_{t}")
                nc.sync.dma_start(xt[:], x_p[b, t * 128:(t + 1) * 128, :])
                st = lnpool.tile([128, 6], f32, tag="st", name=f"st{b}_{t}")
                nc.vector.bn_stats(st[:], xt[:])
                mv = lnpool.tile([128, 2], f32, tag="mv", name=f"mv{b}_{t}")
                nc.vector.bn_aggr(mv[:], st[:])
                sd = lnpool.tile([128, 1], f32, tag="sd", name=f"sd{b}_{t}")
                nc.scalar.activation(sd[:], mv[:, 1:2], Sqrt, bias=epst[:])
                rs = lnpool.tile([128, 1], f32, tag="rs", name=f"rs{b}_{t}")
                nc.vector.reciprocal(rs[:], sd[:])
                xn = lnpool.tile([128, E], f32, tag="xnt", name=f"xn{b}_{t}")
                nc.vector.tensor_scalar(
                    xn[:], xt[:], mv[:, 0:1], rs[:], op0=sub, op1=mult
                )
                tp = psm.tile([128, E], f32, tag="m512", name=f"lntp{b}_{t}")
                nc.tensor.transpose(tp[:], xn[:], ident[:])
                nc.vector.tensor_scalar(
                    xnT[b][:, t * 128:(t + 1) * 128], tp[:],
                    lnw[:], lnb[:], op0=mult, op1=add,
                )
                for c in range(2):
                    vps = psm.tile([128, 512], f32, tag="m512",
                                   name=f"vps{b}_{t}_{c}")
                    nc.tensor.matmul(
                        vps[:],
                        xnT[b][:, t * 128:(t + 1) * 128],
                        wvf[:, c * 512:(c + 1) * 512],
                        start=True, stop=True,
                    )
                    nc.vector.tensor_copy(
                        vall[b][:, t, c * 512:(c + 1) * 512], vps[:]
                    )

            for t in range(NT):
                ln_and_v(0, t)

            # ---- attention, software-pipelined across (batch, head) ----
            # 3-deep pipeline, emission order per step `it`:
            #   tT(it+1) early (its DVE cast settles before step it+1)
            #   dots/mask/exp/transpose(it) interleaved with PV(it-1)
            #   proj(it-2) last (oT cast from step it-1 has settled)
            iters = [(b, h) for b in range(B_LOC) for h in range(H)]
            NIT = len(iters)
            stash = {}

            def make_tT(it):
                b, h = iters[it]
                tT = wpool.tile([E, N], bf16, tag="tT", name=f"tT{it}")
                for qc in range(2):
                    tps = psm.tile([128, 512], f32, tag="m512", name=f"tps{it}_{qc}")
                    nc.tensor.matmul(
                        tps[:], amat[:, h, :],
                        xnT[b][:, qc * 512:(qc + 1) * 512],
                        start=True, stop=True,
                    )
                    nc.vector.tensor_copy(tT[:, qc * 512:(qc + 1) * 512], tps[:])
                stash[("tT", it)] = tT

            def dots_group(it, qts, P, rsum, PT):
                b, h = iters[it]
                tT = stash[("tT", it)]
                for qt in qts:
                    dps = psd.tile([128, N], f32, tag="dots", name=f"dps{it}_{qt}")
                    for kc in range(2):
                        nc.tensor.matmul(
                            dps[:, kc * 512:(kc + 1) * 512],
                            tT[:, qt * 128:(qt + 1) * 128],
                            xnT[b][:, kc * 512:(kc + 1) * 512],
                            start=True, stop=True,
                        )
                    nc.vector.tensor_add(
                        dps[:, qt * 128:(qt + 1) * 128],
                        dps[:, qt * 128:(qt + 1) * 128],
                        dmask[:],
                    )
                    nc.scalar.activation(
                        P[:, qt, :], dps[:], Exp, bias=zbias[:],
                        accum_out=rsum[:, qt:qt + 1],
                    )
                    nc.sync.dma_start(
                        out=PT[:, :, qt * 128:(qt + 1) * 128],
                        in_=P[:, qt, :],
                        transpose=True,
                    )

            def pv_chain(it, qc):
                b, h = iters[it]
                PT = stash[("PT", it)]
                oT = stash[("oT", it)]
                ops = psm.tile([128, 512], f32, tag="m512", name=f"ops{it}_{qc}")
                for kt in range(NT):
                    nc.tensor.matmul(
                        ops[:],
                        vall[b][:, kt, h * E:(h + 1) * E],
                        PT[:, kt, qc * 512:(qc + 1) * 512],
                        start=(kt == 0), stop=(kt == NT - 1),
                    )
                nc.vector.tensor_copy(oT[:, qc * 512:(qc + 1) * 512], ops[:])

            def proj(it):
                b, h = iters[it]
                oT = stash.pop(("oT", it))
                rcp = stash.pop(("rcp", it))
                stash.pop(("tT", it))
                stash.pop(("PT", it))
                for t in range(NT):
                    yps = psm.tile([128, E], f32, tag="m512", name=f"yps{it}_{t}")
                    nc.tensor.matmul(
                        yps[:],
                        oT[:, t * 128:(t + 1) * 128],
                        wp[:, h, :],
                        start=True, stop=True,
                    )
                    if h == 0:
                        nc.vector.scalar_tensor_tensor(
                            y_acc[b][:, t * 128:(t + 1) * 128],
                            yps[:], rcp[:, t:t + 1], bptile[:],
                            op0=mult, op1=add,
                        )
                    else:
                        nc.vector.scalar_tensor_tensor(
                            y_acc[b][:, t * 128:(t + 1) * 128],
                            yps[:], rcp[:, t:t + 1],
                            y_acc[b][:, t * 128:(t + 1) * 128],
                            op0=mult, op1=add,
                        )
                if h == H - 1:
                    for t2 in range(0, NT, 2):
                        nc.scalar.dma_start(
                            out_p[b, t2 * 128:(t2 + 2) * 128].rearrange(
                                "(t p) e -> p t e", p=128),
                            y_acc[b][:, t2 * 128:(t2 + 2) * 128].rearrange(
                                "p (t e) -> p t e", t=2),
                        )


            make_tT(0)
            for it in range(NIT + 2):
                cur = it if it < NIT else None
                if cur is not None:
                    b, h = iters[cur]
                    P = ppool.tile([128, NT, N], bf16, tag="P", name=f"P{cur}")
                    rsum = wpool.tile([128, NT], f32, tag="rsum", name=f"rsum{cur}")
                    PT = ptpool.tile([128, NT, N], bf16, tag="PT", name=f"PT{cur}")
                    oT = wpool.tile([E, N], bf16, tag="oT", name=f"oT{cur}")
                    stash[("PT", cur)] = PT
                    stash[("oT", cur)] = oT
                    dots_group(cur, [0, 1, 2, 3], P, rsum, PT)
                if it - 1 >= 0 and it - 1 < NIT:
                    pv_chain(it - 1, 0)
                if cur is not None:
                    dots_group(cur, [4, 5, 6, 7], P, rsum, PT)
                if it - 1 >= 0 and it - 1 < NIT:
                    pv_chain(it - 1, 1)
                if cur is not None:
                    rcp = wpool.tile([128, NT], f32, tag="rcp", name=f"rcp{cur}")
                    nc.vector.reciprocal(rcp[:], rsum[:])
                    stash[("rcp", cur)] = rcp
                if cur is not None and cur + 1 < NIT:
                    make_tT(cur + 1)
                if 1 <= it <= 4:
                    ln_and_v(1, (it - 1) * 2)
                    ln_and_v(1, (it - 1) * 2 + 1)
                if it - 2 >= 0 and it - 2 < NIT:
                    proj(it - 2)

    nc.compile()
    return nc


def _get_nc():
    if "nc" not in _cache:
        _cache["nc"] = _build_nc()
    return _cache["nc"]


def _make_in_maps(inputs):
    x = np.ascontiguousarray(np.asarray(inputs["x"], dtype=np.float32))
    ln_w = np.asarray(inputs["ln_w"], dtype=np.float32)
    ln_b = np.asarray(inputs["ln_b"], dtype=np.float32)
    w_qkv = np.asarray(inputs["w_qkv"], dtype=np.float32)
    scale = np.asarray(inputs["scale"], dtype=np.float32)
    w_proj = np.asarray(inputs["w_proj"], dtype=np.float32)

    INNER = E * H
    Wq = w_qkv[:, :INNER]
    Wk = w_qkv[:, INNER:2 * INNER]
    Wv = w_qkv[:, 2 * INNER:]

    amat = np.stack(
        [scale[h] * (Wq[:, h * E:(h + 1) * E] @ Wk[:, h * E:(h + 1) * E].T)
         for h in range(H)]
    ).astype(ml_dtypes.bfloat16)  # [H, E, E]
    wvf = Wv.astype(ml_dtypes.bfloat16)  # [E, INNER]
    wp = w_proj.reshape(H, E, E).astype(ml_dtypes.bfloat16)  # [H, d, E]
    ident = np.eye(E, dtype=np.float32)
    dmask = (np.eye(E, dtype=np.float32) * MASK_VAL).astype(np.float32)
    lnw = ln_w.reshape(E, 1)
    lnb = ln_b.reshape(E, 1)

    b_proj = np.asarray(inputs["b_proj"], dtype=np.float32)
    bptile = np.broadcast_to(b_proj[None, :], (128, E)).copy()
    shared = {
        "amat": amat, "wvf": wvf, "wp": wp, "bptile": bptile,
        "lnw": lnw, "lnb": lnb, "ident": ident, "dmask": dmask,
    }
    return [
        {"x": x[c * B_LOC:(c + 1) * B_LOC], **shared} for c in range(NCORES)
    ]


def kernel(x, ln_w, ln_b, w_qkv, scale, w_proj, b_proj):
    from concourse.bass_utils import run_bass_kernel_spmd

    in_maps = _make_in_maps(dict(
        x=x, ln_w=ln_w, ln_b=ln_b, w_qkv=w_qkv, scale=scale,
        w_proj=w_proj, b_proj=b_proj,
    ))
    b_proj = np.asarray(b_proj, dtype=np.float32)

    nc = _get_nc()
    res = run_bass_kernel_spmd(nc, in_maps, core_ids=list(range(NCORES)))
    y = np.concatenate([res.results[c]["out"] for c in range(NCORES)], axis=0)
    return y.astype(np.float32)
